# revision 3
# baseline (speedup 1.0000x reference)
# Cross-attention SDPA kernel for 8 Trainium2 NeuronCores.
#
# reference semantics (per batch b):
#   Q = y @ Wq + bq            [N, 64]
#   K = z @ Wk + bk            [M, 64]
#   V = z @ Wv + bv            [M, 64]
#   O = softmax(Q K^T / 8) V   [N, 64]
# B=4, M=N=4096, D=512.
#
# Sharding: 8 cores = 4 batches x 2 halves of the query (decoder) length.
# Each core sees z^T[b] (full, [512,4096]) and y^T half ([512,2048]),
# pre-transposed and cast to bf16 on the host, and produces O rows
# [2048, 64] fp32.
#
# On-core dataflow (S^T layout so the softmax reduction rides the matmul):
#   zt/yt      d on partitions, 4 chunks of 128
#   K^T        [64,4096], stored [128,512] per block: even m-tiles on
#              partitions 0:64, odd on 64:128 -> 2-way row-packed S matmuls
#   Q^T        [64,2048] duplicated to partitions 64:128
#   V          [m,64] natural + ones column (softmax denominator)
#   S^T pair   = K^T-tile.T @ Q^T  (two concurrent row-group matmuls)
#   E          = exp(S^T * 0.125)  (ScalarE, PSUM -> SBUF bf16)
#   O^T[65,n] += V_aug.T @ E       (row 64 = denominator)
#   O[n,64]    = transpose(O^T) * (1/denom) + bv  -> DRAM
#
# bq/bk fold into the PSUM->SBUF projection copies as per-partition bias; bv
# is added at the end (softmax rows sum to 1 so P @ (1 bv^T) == 1 bv^T).

import numpy as np
import ml_dtypes

B = 4
M = 4096
N = 4096
D = 512
KQ = 64
NH = N // 2          # per-core query rows
NCORES = 8
SCALE = 0.125        # 1/sqrt(64)

NT = M // 128        # 32 m-tiles
NPAIR = NT // 2      # 16 pairs of adjacent m-tiles (2p, 2p+1)
NBLK = 4             # n-blocks of 512 in the attention loop
QBLK = NH // 512     # 4 q-projection blocks
KBLK = M // 512      # 8 k-projection blocks

_CACHE = {}


def build_program():
    from contextlib import ExitStack

    import concourse.bacc as bacc
    import concourse.mybir as mybir
    import concourse.tile as tile
    from concourse.bass import ts, ds
    from concourse.masks import make_identity

    F32 = mybir.dt.float32
    BF16 = mybir.dt.bfloat16
    EXP = mybir.ActivationFunctionType.Exp

    nc = bacc.Bacc("TRN2", target_bir_lowering=False, debug=False)

    zt_d = nc.dram_tensor("zt", [4, 128, M], BF16, kind="ExternalInput").ap()
    yt_d = nc.dram_tensor("yt", [4, 128, NH], BF16, kind="ExternalInput").ap()
    wq_d = nc.dram_tensor("wq", [4, 128, KQ], BF16, kind="ExternalInput").ap()
    wk_d = nc.dram_tensor("wk", [4, 128, KQ], BF16, kind="ExternalInput").ap()
    wv_d = nc.dram_tensor("wv", [4, 128, KQ], BF16, kind="ExternalInput").ap()
    bq_d = nc.dram_tensor("bq", [KQ, 1], F32, kind="ExternalInput").ap()
    bk_d = nc.dram_tensor("bk", [KQ, 1], F32, kind="ExternalInput").ap()
    bv_d = nc.dram_tensor("bv", [1, KQ], F32, kind="ExternalInput").ap()
    o_d = nc.dram_tensor("o", [NH, KQ], F32, kind="ExternalOutput").ap()

    with ExitStack() as ctx:
        tc = ctx.enter_context(tile.TileContext(nc))
        singles = ctx.enter_context(tc.tile_pool(name="singles", bufs=1))
        epool = ctx.enter_context(tc.tile_pool(name="epool", bufs=3))
        otpool = ctx.enter_context(tc.tile_pool(name="otpool", bufs=2))
        ostage = ctx.enter_context(tc.tile_pool(name="ostage", bufs=3))
        rpool = ctx.enter_context(tc.tile_pool(name="rpool", bufs=3))
        spool = ctx.enter_context(tc.tile_pool(name="spool", bufs=2, space="PSUM"))
        opool = ctx.enter_context(tc.tile_pool(name="opool", bufs=2, space="PSUM"))
        ppool = ctx.enter_context(tc.tile_pool(name="ppool", bufs=1, space="PSUM"))
        fpool = ctx.enter_context(tc.tile_pool(name="fpool", bufs=1, space="PSUM"))

        # --- constants / small inputs ---
        ident = singles.tile([128, 128], F32, name="ident", tag="ident")
        make_identity(nc, ident)
        bq_sb = singles.tile([KQ, 1], F32, name="bq", tag="bq")
        nc.sync.dma_start(bq_sb, bq_d)
        bk_sb = singles.tile([128, 1], F32, name="bk", tag="bk")
        nc.sync.dma_start(bk_sb[0:64, :], bk_d)
        nc.sync.dma_start(bk_sb[64:128, :], bk_d)
        bv_sb = singles.tile([128, KQ], F32, name="bv", tag="bv")
        nc.sync.dma_start(bv_sb, bv_d.to_broadcast((128, KQ)))

        # warm the exp table while DMAs stream
        warm = singles.tile([64, 1], F32, name="warm", tag="warm")
        nc.scalar.activation(warm, bq_sb, EXP, scale=1.0)

        wq_sb = []
        wk_sb = []
        wv_sb = []
        for c in range(4):
            wq_sb.append(singles.tile([128, KQ], BF16, name=f"wq{c}", tag=f"wq{c}"))
            nc.sync.dma_start(wq_sb[c], wq_d[c])
            wk_sb.append(singles.tile([128, KQ], BF16, name=f"wk{c}", tag=f"wk{c}"))
            nc.sync.dma_start(wk_sb[c], wk_d[c])
            wv_sb.append(singles.tile([128, KQ], BF16, name=f"wv{c}", tag=f"wv{c}"))
            nc.sync.dma_start(wv_sb[c], wv_d[c])

        # --- activation SBUF tensors (filled blockwise below) ---
        yt = [
            singles.tile([128, NH], BF16, name=f"yt{c}", tag=f"yt{c}")
            for c in range(4)
        ]
        zt = [
            [
                singles.tile([128, 2048], BF16, name=f"zt{c}{h}", tag=f"zt{c}{h}")
                for h in range(2)
            ]
            for c in range(4)
        ]
        qt_blk = [
            singles.tile([128, 512], BF16, name=f"qt{j}", tag=f"qt{j}")
            for j in range(QBLK)
        ]
        kt_blk = [
            singles.tile([128, 512], BF16, name=f"kt{j}", tag=f"kt{j}")
            for j in range(4)
        ]
        v_sb = [
            singles.tile([128, KQ + 1], BF16, name=f"v{t}", tag=f"v{t}")
            for t in range(NT)
        ]

        def dma_z_block(b):
            h, bb = divmod(b, 4)
            for c in range(4):
                nc.sync.dma_start(
                    zt[c][h][:, ds(bb * 512, 512)],
                    zt_d[c, :, ds(b * 512, 512)],
                )

        def dma_y_block(j):
            for c in range(4):
                nc.sync.dma_start(
                    yt[c][:, ds(j * 512, 512)], yt_d[c, :, ds(j * 512, 512)]
                )

        def q_proj(j):
            q_ps = ppool.tile([128, 512], F32, name="proj", tag="proj")
            for c in range(4):
                nc.tensor.matmul(
                    q_ps[0:64, :],
                    lhsT=wq_sb[c],
                    rhs=yt[c][:, ts(j, 512)],
                    start=(c == 0),
                    stop=(c == 3),
                )
            nc.vector.tensor_scalar_add(qt_blk[j][0:64, :], q_ps[0:64, :], bq_sb)
            nc.sync.dma_start(qt_blk[j][64:128, :], qt_blk[j][0:64, :])

        def k_proj(b):
            # m-block b holds m-tiles 4b..4b+3; even tiles go to partitions
            # 0:64 of kt_blk[b//2] cols (b%2)*256.., odd tiles to 64:128.
            j, seg = divmod(b, 2)
            h = b // 4
            k_ps = ppool.tile([128, 512], F32, name="proj", tag="proj")
            views = [
                zt[c][h].rearrange("p (t x) -> p t x", x=128) for c in range(4)
            ]
            t0 = 4 * (b % 4)
            for c in range(4):
                nc.tensor.matmul(
                    k_ps[0:64, 0:256],
                    lhsT=wk_sb[c],
                    rhs=views[c][:, t0 : t0 + 4 : 2, :],
                    start=(c == 0),
                    stop=(c == 3),
                    tile_position=(0, 0),
                )
            for c in range(4):
                nc.tensor.matmul(
                    k_ps[64:128, 0:256],
                    lhsT=wk_sb[c],
                    rhs=views[c][:, t0 + 1 : t0 + 4 : 2, :],
                    start=(c == 0),
                    stop=(c == 3),
                    tile_position=(0, 64),
                )
            nc.vector.tensor_scalar_add(
                kt_blk[j][0:64, ds(seg * 256, 256)],
                k_ps[0:64, 0:256],
                bk_sb[0:64, :],
            )
            nc.vector.tensor_scalar_add(
                kt_blk[j][64:128, ds(seg * 256, 256)],
                k_ps[64:128, 0:256],
                bk_sb[64:128, :],
            )

        def v_proj(t):
            h, tt = divmod(t, 16)
            v_ps = ppool.tile([128, 512], F32, name="proj", tag="proj")
            for c in range(4):
                nc.tensor.matmul(
                    v_ps[:, 0:KQ],
                    lhsT=zt[c][h][:, ts(tt, 128)],
                    rhs=wv_sb[c],
                    start=(c == 0),
                    stop=(c == 3),
                )
            nc.vector.tensor_copy(v_sb[t][:, 0:KQ], v_ps[:, 0:KQ])
            nc.vector.memset(v_sb[t][:, KQ : KQ + 1], 1.0)

        # --- emission order chosen so the attention loop can start after the
        # first y/z blocks land; the rest streams in underneath it ---
        dma_y_block(0)
        dma_z_block(0)
        q_proj(0)
        k_proj(0)
        for t in range(4):
            v_proj(t)
        for b in range(1, KBLK):
            dma_z_block(b)
            if b < 4:
                dma_y_block(b)
                q_proj(b)
            k_proj(b)
            for t in range(4 * b, 4 * b + 4):
                v_proj(t)

        # --- attention loop ---
        for nb in range(NBLK):
            o_ps = opool.tile([128, 512], F32, name="o", tag="o")
            qlo = qt_blk[nb][0:64, :]
            qhi = qt_blk[nb][64:128, :]
            for p in range(NPAIR):
                jj, col = divmod(p, 4)
                s_ps = spool.tile([128, 1024], F32, name="s", tag="s")
                nc.tensor.matmul(
                    s_ps[:, 0:512],
                    lhsT=kt_blk[jj][0:64, ts(col, 128)],
                    rhs=qlo,
                    start=True,
                    stop=True,
                    tile_position=(0, 0),
                )
                nc.tensor.matmul(
                    s_ps[:, 512:1024],
                    lhsT=kt_blk[jj][64:128, ts(col, 128)],
                    rhs=qhi,
                    start=True,
                    stop=True,
                    tile_position=(64, 0),
                )
                e_t = epool.tile([128, 1024], BF16, name="e", tag="e")
                nc.scalar.activation(e_t, s_ps, EXP, scale=SCALE)
                nc.tensor.matmul(
                    o_ps[0:65, :],
                    lhsT=v_sb[2 * p],
                    rhs=e_t[:, 0:512],
                    start=(p == 0),
                    stop=False,
                )
                nc.tensor.matmul(
                    o_ps[0:65, :],
                    lhsT=v_sb[2 * p + 1],
                    rhs=e_t[:, 512:1024],
                    start=False,
                    stop=(p == NPAIR - 1),
                )

            # --- finalize this n-block: transpose, normalize, bias, store ---
            ot_sb = otpool.tile([128, 512], F32, name="ot", tag="ot")
            nc.vector.tensor_copy(ot_sb[0:65, :], o_ps[0:65, :])
            for s in range(4):
                ot_ps = fpool.tile([128, 512], F32, name="fin", tag="fin")
                nc.tensor.matmul(
                    ot_ps[:, 0:65],
                    lhsT=ot_sb[0:65, ts(s, 128)],
                    rhs=ident[0:65, 0:65],
                    is_transpose=True,
                    start=True,
                    stop=True,
                )
                rcp = rpool.tile([128, 1], F32, name="rcp", tag="rcp")
                nc.vector.reciprocal(rcp, ot_ps[:, 64:65])
                o_st = ostage.tile([128, KQ], F32, name="ost", tag="ost")
                nc.vector.tensor_scalar_mul(o_st, ot_ps[:, 0:KQ], rcp)
                nc.vector.tensor_add(o_st, o_st, bv_sb)
                nc.sync.dma_start(o_d[ds(nb * 512 + s * 128, 128), :], o_st)

    nc.compile()
    return nc


def _get_program():
    if "nc" not in _CACHE:
        _CACHE["nc"] = build_program()
    return _CACHE["nc"]


def make_in_maps(z, y, Wq, bq, Wk, bk, Wv, bv):
    bf16 = ml_dtypes.bfloat16
    zt = np.ascontiguousarray(z.astype(bf16).transpose(0, 2, 1))  # [B, 512, M]
    yt = np.ascontiguousarray(y.astype(bf16).transpose(0, 2, 1))  # [B, 512, N]
    wq = np.ascontiguousarray(Wq.astype(bf16).reshape(4, 128, KQ))
    wk = np.ascontiguousarray(Wk.astype(bf16).reshape(4, 128, KQ))
    wv = np.ascontiguousarray(Wv.astype(bf16).reshape(4, 128, KQ))
    bq2 = np.ascontiguousarray(bq.astype(np.float32).reshape(KQ, 1))
    bk2 = np.ascontiguousarray(bk.astype(np.float32).reshape(KQ, 1))
    bv2 = np.ascontiguousarray(bv.astype(np.float32).reshape(1, KQ))
    in_maps = []
    for c in range(NCORES):
        b, h = divmod(c, 2)
        in_maps.append(
            {
                "zt": zt[b].reshape(4, 128, M),
                "yt": np.ascontiguousarray(
                    yt[b][:, h * NH : (h + 1) * NH]
                ).reshape(4, 128, NH),
                "wq": wq,
                "wk": wk,
                "wv": wv,
                "bq": bq2,
                "bk": bk2,
                "bv": bv2,
            }
        )
    return in_maps


def kernel(z, y, Wq, bq, Wk, bk, Wv, bv):
    from concourse import bass_utils

    nc = _get_program()
    in_maps = make_in_maps(z, y, Wq, bq, Wk, bk, Wv, bv)
    res = bass_utils.run_bass_kernel_spmd(nc, in_maps, core_ids=list(range(NCORES)))
    out = np.empty((B, N, KQ), dtype=np.float32)
    for c in range(NCORES):
        b, h = divmod(c, 2)
        out[b, h * NH : (h + 1) * NH, :] = res.results[c]["o"]
    return out


# revision 7
# speedup vs baseline: 1.0225x; 1.0225x over previous
# Cross-attention SDPA kernel for 8 Trainium2 NeuronCores.
#
# reference semantics (per batch b):
#   Q = y @ Wq + bq            [N, 64]
#   K = z @ Wk + bk            [M, 64]
#   V = z @ Wv + bv            [M, 64]
#   O = softmax(Q K^T / 8) V   [N, 64]
# B=4, M=N=4096, D=512.
#
# Sharding: 8 cores = 4 batches x 2 halves of the query (decoder) length.
# Each core gets z^T[b] (full, [512,4096]) and its y^T half ([512,2048]),
# pre-transposed and cast to bf16 on the host, and produces O rows
# [2048, 64] fp32.
#
# On-core dataflow (S^T layout so the softmax reduction rides the matmul):
#   zt/yt        d on partitions, 4 chunks of 128
#   fused proj   stationary [wk|wv] (even blocks) / [wv|wk] (odd): one pass
#                over z yields K^T and V^T together; [wq|wq] duplicates Q^T
#                across both partition halves for free
#   K^T          kt_blk[j] [128,512]: tiles 8j..8j+3 on partitions 0:64,
#                tiles 8j+4..8j+7 on 64:128 -> 2-way row-packed S matmuls
#   V            V^T transposed tile-wise on the PE (+ones column appended)
#   S^T pair     two concurrent row-group matmuls (tiles 8j+i, 8j+4+i)
#   E            = exp(S^T * 0.125)  (ScalarE, PSUM -> SBUF bf16)
#   O^T[65,n]   += V_aug.T @ E       (row 64 = softmax denominator)
#   O[n,64]      = transpose(O^T) * (1/denom) + bv  -> DRAM
#
# bq/bk fold into the PSUM->SBUF projection copies as per-partition bias; bv
# is added at the end (softmax rows sum to 1 so P @ (1 bv^T) == 1 bv^T).

import numpy as np
import ml_dtypes

B = 4
M = 4096
N = 4096
D = 512
KQ = 64
NH = N // 2          # per-core query rows
NCORES = 8
SCALE = 0.125        # 1/sqrt(64)

NT = M // 128        # 32 m-tiles
NPAIR = NT // 2      # 16 pairs (8j+i, 8j+4+i)
NBLK = 4             # n-blocks of 512 in the attention loop
QBLK = NH // 512     # 4 q-projection blocks
KBLK = M // 512      # 8 fused kv-projection blocks

_CACHE = {}


def build_program():
    from contextlib import ExitStack

    import concourse.bacc as bacc
    import concourse.mybir as mybir
    import concourse.tile as tile
    from concourse.bass import ts, ds
    from concourse.masks import make_identity

    F32 = mybir.dt.float32
    BF16 = mybir.dt.bfloat16
    EXP = mybir.ActivationFunctionType.Exp

    nc = bacc.Bacc("TRN2", target_bir_lowering=False, debug=False)

    zt_d = nc.dram_tensor("zt", [4, 128, M], BF16, kind="ExternalInput").ap()
    yt_d = nc.dram_tensor("yt", [4, 128, NH], BF16, kind="ExternalInput").ap()
    wq_d = nc.dram_tensor("wq", [4, 128, KQ], BF16, kind="ExternalInput").ap()
    wk_d = nc.dram_tensor("wk", [4, 128, KQ], BF16, kind="ExternalInput").ap()
    wv_d = nc.dram_tensor("wv", [4, 128, KQ], BF16, kind="ExternalInput").ap()
    bq_d = nc.dram_tensor("bq", [KQ, 1], F32, kind="ExternalInput").ap()
    bk_d = nc.dram_tensor("bk", [KQ, 1], F32, kind="ExternalInput").ap()
    bv_d = nc.dram_tensor("bv", [1, KQ], F32, kind="ExternalInput").ap()
    o_d = nc.dram_tensor("o", [NH, KQ], F32, kind="ExternalOutput").ap()

    with ExitStack() as ctx:
        tc = ctx.enter_context(tile.TileContext(nc))
        singles = ctx.enter_context(tc.tile_pool(name="singles", bufs=1))
        epool = ctx.enter_context(tc.tile_pool(name="epool", bufs=3))
        vtpool = ctx.enter_context(tc.tile_pool(name="vtpool", bufs=2))
        otpool = ctx.enter_context(tc.tile_pool(name="otpool", bufs=2))
        ostage = ctx.enter_context(tc.tile_pool(name="ostage", bufs=3))
        rpool = ctx.enter_context(tc.tile_pool(name="rpool", bufs=3))
        spool = ctx.enter_context(tc.tile_pool(name="spool", bufs=2, space="PSUM"))
        opool = ctx.enter_context(tc.tile_pool(name="opool", bufs=2, space="PSUM"))
        ppool = ctx.enter_context(tc.tile_pool(name="ppool", bufs=2, space="PSUM"))

        # --- constants / small inputs ---
        ident = singles.tile([128, 128], F32, name="ident", tag="ident")
        make_identity(nc, ident)
        identb = singles.tile([128, 128], BF16, name="identb", tag="identb")
        make_identity(nc, identb)
        bq_sb = singles.tile([128, 1], F32, name="bq", tag="bq")
        nc.sync.dma_start(bq_sb[0:64, :], bq_d)
        nc.sync.dma_start(bq_sb[64:128, :], bq_d)
        bk_sb = singles.tile([128, 1], F32, name="bk", tag="bk")
        nc.sync.dma_start(bk_sb[0:64, :], bk_d)
        nc.sync.dma_start(bk_sb[64:128, :], bk_d)
        bv_sb = singles.tile([128, KQ], F32, name="bv", tag="bv")
        nc.sync.dma_start(bv_sb, bv_d.to_broadcast((128, KQ)))

        # warm the exp table while DMAs stream
        warm = singles.tile([64, 1], F32, name="warm", tag="warm")
        nc.scalar.activation(warm, bq_sb[0:64, :], EXP, scale=1.0)

        # fused projection weights: [wq|wq], [wk|wv] (even), [wv|wk] (odd)
        wqq = []
        wkv_e = []
        wkv_o = []
        for c in range(4):
            t = singles.tile([128, 128], BF16, name=f"wqq{c}", tag=f"wqq{c}")
            nc.sync.dma_start(t[:, 0:KQ], wq_d[c])
            nc.sync.dma_start(t[:, KQ:128], wq_d[c])
            wqq.append(t)
            t = singles.tile([128, 128], BF16, name=f"wkve{c}", tag=f"wkve{c}")
            nc.sync.dma_start(t[:, 0:KQ], wk_d[c])
            nc.sync.dma_start(t[:, KQ:128], wv_d[c])
            wkv_e.append(t)
            t = singles.tile([128, 128], BF16, name=f"wkvo{c}", tag=f"wkvo{c}")
            nc.sync.dma_start(t[:, 0:KQ], wv_d[c])
            nc.sync.dma_start(t[:, KQ:128], wk_d[c])
            wkv_o.append(t)

        # --- activation SBUF tensors (filled blockwise below) ---
        yt = [
            singles.tile([128, NH], BF16, name=f"yt{c}", tag=f"yt{c}")
            for c in range(4)
        ]
        zt = [
            [
                singles.tile([128, 2048], BF16, name=f"zt{c}{h}", tag=f"zt{c}{h}")
                for h in range(2)
            ]
            for c in range(4)
        ]
        qt_blk = [
            singles.tile([128, 512], BF16, name=f"qt{j}", tag=f"qt{j}")
            for j in range(QBLK)
        ]
        kt_blk = [
            singles.tile([128, 512], BF16, name=f"kt{j}", tag=f"kt{j}")
            for j in range(4)
        ]
        v_sb = [
            singles.tile([128, KQ + 1], BF16, name=f"v{t}", tag=f"v{t}")
            for t in range(NT)
        ]

        def dma_z_block(b):
            h, bb = divmod(b, 4)
            for c in range(4):
                nc.sync.dma_start(
                    zt[c][h][:, ds(bb * 512, 512)],
                    zt_d[c, :, ds(b * 512, 512)],
                )

        def dma_y_block(j):
            for c in range(4):
                nc.sync.dma_start(
                    yt[c][:, ds(j * 512, 512)], yt_d[c, :, ds(j * 512, 512)]
                )

        def q_proj(j):
            # [wq|wq] stationary -> Q^T appears on both partition halves
            q_ps = ppool.tile([128, 512], F32, name="proj", tag="proj")
            for c in range(4):
                nc.tensor.matmul(
                    q_ps,
                    lhsT=wqq[c],
                    rhs=yt[c][:, ts(j, 512)],
                    start=(c == 0),
                    stop=(c == 3),
                )
            nc.vector.tensor_scalar_add(qt_blk[j], q_ps, bq_sb)

        def kv_proj(b):
            # one pass over z m-block b (tiles 4b..4b+3) produces
            # K^T -> kt_blk[b//2] (even b: partitions 0:64, odd: 64:128)
            # V^T -> staging, then PE-transposed into v_sb tiles
            h = b // 4
            bb = b % 4
            odd = b % 2
            w = wkv_o if odd else wkv_e
            khalf = slice(64, 128) if odd else slice(0, 64)
            vhalf = slice(0, 64) if odd else slice(64, 128)
            kv_ps = ppool.tile([128, 512], F32, name="proj", tag="proj")
            for c in range(4):
                nc.tensor.matmul(
                    kv_ps,
                    lhsT=w[c],
                    rhs=zt[c][h][:, ts(bb, 512)],
                    start=(c == 0),
                    stop=(c == 3),
                )
            nc.vector.tensor_scalar_add(
                kt_blk[b // 2][khalf, :], kv_ps[khalf, :], bk_sb[khalf, :]
            )
            vt_sb = vtpool.tile([128, 512], BF16, name="vt", tag="vt")
            nc.vector.tensor_copy(vt_sb[vhalf, :], kv_ps[vhalf, :])
            vib = identb[64:128, 64:128] if odd == 0 else identb[0:64, 0:64]
            for i in range(4):
                t = 4 * b + i
                v_ps = ppool.tile([128, 512], BF16, name="projb", tag="proj")
                nc.tensor.matmul(
                    v_ps[:, 0:KQ],
                    lhsT=vt_sb[vhalf, ts(i, 128)],
                    rhs=vib,
                    is_transpose=True,
                    start=True,
                    stop=True,
                    tile_position=(vhalf.start, 0),
                )
                nc.vector.tensor_copy(v_sb[t][:, 0:KQ], v_ps[:, 0:KQ])
                nc.vector.memset(v_sb[t][:, KQ : KQ + 1], 1.0)

        # --- emission order: unblock the attention loop asap, stream the
        # rest of the projections underneath it ---
        dma_y_block(0)
        dma_z_block(0)
        dma_z_block(1)
        q_proj(0)
        kv_proj(0)
        kv_proj(1)
        for b in range(2, KBLK):
            dma_z_block(b)
            if b in (2, 4, 6):
                dma_y_block(b // 2)
                q_proj(b // 2)
            kv_proj(b)

        # --- attention loop ---
        for nb in range(NBLK):
            o_ps = opool.tile([128, 512], F32, name="o", tag="o")
            qlo = qt_blk[nb][0:64, :]
            qhi = qt_blk[nb][64:128, :]
            for p in range(NPAIR):
                jj, col = divmod(p, 4)
                s_ps = spool.tile([128, 1024], F32, name="s", tag="s")
                nc.tensor.matmul(
                    s_ps[:, 0:512],
                    lhsT=kt_blk[jj][0:64, ts(col, 128)],
                    rhs=qlo,
                    start=True,
                    stop=True,
                    tile_position=(0, 0),
                )
                nc.tensor.matmul(
                    s_ps[:, 512:1024],
                    lhsT=kt_blk[jj][64:128, ts(col, 128)],
                    rhs=qhi,
                    start=True,
                    stop=True,
                    tile_position=(64, 0),
                )
                e_t = epool.tile([128, 1024], BF16, name="e", tag="e")
                nc.scalar.activation(e_t, s_ps, EXP, scale=SCALE)
                nc.tensor.matmul(
                    o_ps[0:65, :],
                    lhsT=v_sb[8 * jj + col],
                    rhs=e_t[:, 0:512],
                    start=(p == 0),
                    stop=False,
                )
                nc.tensor.matmul(
                    o_ps[0:65, :],
                    lhsT=v_sb[8 * jj + 4 + col],
                    rhs=e_t[:, 512:1024],
                    start=False,
                    stop=(p == NPAIR - 1),
                )

            # --- finalize this n-block: transpose, normalize, bias, store ---
            ot_sb = otpool.tile([128, 512], F32, name="ot", tag="ot")
            nc.vector.tensor_copy(ot_sb[0:65, :], o_ps[0:65, :])
            for s in range(4):
                ot_ps = ppool.tile([128, 512], F32, name="proj", tag="proj")
                nc.tensor.matmul(
                    ot_ps[:, 0:65],
                    lhsT=ot_sb[0:65, ts(s, 128)],
                    rhs=ident[0:65, 0:65],
                    is_transpose=True,
                    start=True,
                    stop=True,
                )
                rcp = rpool.tile([128, 1], F32, name="rcp", tag="rcp")
                nc.vector.reciprocal(rcp, ot_ps[:, 64:65])
                o_st = ostage.tile([128, KQ], F32, name="ost", tag="ost")
                nc.vector.tensor_scalar_mul(o_st, ot_ps[:, 0:KQ], rcp)
                nc.vector.tensor_add(o_st, o_st, bv_sb)
                nc.sync.dma_start(o_d[ds(nb * 512 + s * 128, 128), :], o_st)

    nc.compile()
    return nc


def _get_program():
    if "nc" not in _CACHE:
        _CACHE["nc"] = build_program()
    return _CACHE["nc"]


def make_in_maps(z, y, Wq, bq, Wk, bk, Wv, bv):
    bf16 = ml_dtypes.bfloat16
    zt = np.ascontiguousarray(z.astype(bf16).transpose(0, 2, 1))  # [B, 512, M]
    yt = np.ascontiguousarray(y.astype(bf16).transpose(0, 2, 1))  # [B, 512, N]
    wq = np.ascontiguousarray(Wq.astype(bf16).reshape(4, 128, KQ))
    wk = np.ascontiguousarray(Wk.astype(bf16).reshape(4, 128, KQ))
    wv = np.ascontiguousarray(Wv.astype(bf16).reshape(4, 128, KQ))
    bq2 = np.ascontiguousarray(bq.astype(np.float32).reshape(KQ, 1))
    bk2 = np.ascontiguousarray(bk.astype(np.float32).reshape(KQ, 1))
    bv2 = np.ascontiguousarray(bv.astype(np.float32).reshape(1, KQ))
    in_maps = []
    for c in range(NCORES):
        b, h = divmod(c, 2)
        in_maps.append(
            {
                "zt": zt[b].reshape(4, 128, M),
                "yt": np.ascontiguousarray(
                    yt[b][:, h * NH : (h + 1) * NH]
                ).reshape(4, 128, NH),
                "wq": wq,
                "wk": wk,
                "wv": wv,
                "bq": bq2,
                "bk": bk2,
                "bv": bv2,
            }
        )
    return in_maps


def kernel(z, y, Wq, bq, Wk, bk, Wv, bv):
    from concourse import bass_utils

    nc = _get_program()
    in_maps = make_in_maps(z, y, Wq, bq, Wk, bk, Wv, bv)
    res = bass_utils.run_bass_kernel_spmd(nc, in_maps, core_ids=list(range(NCORES)))
    out = np.empty((B, N, KQ), dtype=np.float32)
    for c in range(NCORES):
        b, h = divmod(c, 2)
        out[b, h * NH : (h + 1) * NH, :] = res.results[c]["o"]
    return out


# revision 8
# speedup vs baseline: 1.1141x; 1.0896x over previous
# Cross-attention SDPA kernel for 8 Trainium2 NeuronCores.
#
# reference semantics (per batch b):
#   Q = y @ Wq + bq            [N, 64]
#   K = z @ Wk + bk            [M, 64]
#   V = z @ Wv + bv            [M, 64]
#   O = softmax(Q K^T / 8) V   [N, 64]
# B=4, M=N=4096, D=512.
#
# Sharding: 8 cores = 4 batches x 2 halves of the query (decoder) length.
# Each core gets z^T[b] (full, [512,4096]) and its y^T half ([512,2048]),
# pre-transposed and cast to bf16 on the host, and produces O rows
# [2048, 64] fp32.
#
# On-core dataflow (S^T layout so the softmax reduction rides the matmul):
#   zt/yt        d on partitions, 4 chunks of 128; DMA issue cost (~0.6us per
#                dma_start on the issuing sequencer) is spread over the three
#                DMA-capable engines (sync/scalar HWDGE + gpsimd SWDGE)
#   fused proj   stationary [wk|wv] (even blocks) / [wv|wk] (odd): one pass
#                over z yields K^T and V^T together; [wq|wq] duplicates Q^T
#                across both partition halves for free
#   K^T          kt_blk[j] [128,512]: tiles 8j..8j+3 on partitions 0:64,
#                tiles 8j+4..8j+7 on 64:128 -> 2-way row-packed S matmuls
#   V            V^T transposed tile-wise on the PE (+ones column appended)
#   S^T pair     two concurrent row-group matmuls (tiles 8j+i, 8j+4+i)
#   E            = exp(S^T * 0.125)  (ScalarE, PSUM -> SBUF bf16)
#   O^T[65,n]   += V_aug.T @ E       (row 64 = softmax denominator)
#   O[n,64]      = transpose(O^T) * (1/denom) + bv  -> DRAM
#
# bq/bk fold into the PSUM->SBUF projection copies as per-partition bias; bv
# is added at the end (softmax rows sum to 1 so P @ (1 bv^T) == 1 bv^T).

import numpy as np
import ml_dtypes

B = 4
M = 4096
N = 4096
D = 512
KQ = 64
NH = N // 2          # per-core query rows
NCORES = 8
SCALE = 0.125        # 1/sqrt(64)

NT = M // 128        # 32 m-tiles
NPAIR = NT // 2      # 16 pairs (8j+i, 8j+4+i)
NBLK = 4             # n-blocks of 512 in the attention loop
QBLK = NH // 512     # 4 q-projection blocks
KBLK = M // 512      # 8 fused kv-projection blocks

_CACHE = {}


def build_program():
    from contextlib import ExitStack

    import concourse.bacc as bacc
    import concourse.mybir as mybir
    import concourse.tile as tile
    from concourse.bass import ts, ds
    from concourse.masks import make_identity

    F32 = mybir.dt.float32
    BF16 = mybir.dt.bfloat16
    EXP = mybir.ActivationFunctionType.Exp
    MULT = mybir.AluOpType.mult
    ADD = mybir.AluOpType.add

    nc = bacc.Bacc("TRN2", target_bir_lowering=False, debug=False)

    zt_d = nc.dram_tensor("zt", [4, 128, M], BF16, kind="ExternalInput").ap()
    yt_d = nc.dram_tensor("yt", [4, 128, NH], BF16, kind="ExternalInput").ap()
    # 12 fused weight tiles: [wq|wq]x4, [wk|wv]x4, [wv|wk]x4
    wp_d = nc.dram_tensor("wpack", [128, 1536], BF16, kind="ExternalInput").ap()
    # col 0: bq (dup both halves), col 1: bk (dup), cols 2:66: bv broadcast
    bp_d = nc.dram_tensor("bpack", [128, 66], F32, kind="ExternalInput").ap()
    o_d = nc.dram_tensor("o", [NH, KQ], F32, kind="ExternalOutput").ap()

    with ExitStack() as ctx:
        tc = ctx.enter_context(tile.TileContext(nc))
        singles = ctx.enter_context(tc.tile_pool(name="singles", bufs=1))
        epool = ctx.enter_context(tc.tile_pool(name="epool", bufs=3))
        vtpool = ctx.enter_context(tc.tile_pool(name="vtpool", bufs=2))
        otpool = ctx.enter_context(tc.tile_pool(name="otpool", bufs=2))
        ostage = ctx.enter_context(tc.tile_pool(name="ostage", bufs=3))
        rpool = ctx.enter_context(tc.tile_pool(name="rpool", bufs=3))
        spool = ctx.enter_context(tc.tile_pool(name="spool", bufs=2, space="PSUM"))
        opool = ctx.enter_context(tc.tile_pool(name="opool", bufs=2, space="PSUM"))
        ppool = ctx.enter_context(tc.tile_pool(name="ppool", bufs=2, space="PSUM"))

        # --- constants ---
        wpack = singles.tile([128, 1536], BF16, name="wpack", tag="wpack")
        nc.sync.dma_start(wpack, wp_d)
        bpack = singles.tile([128, 66], F32, name="bpack", tag="bpack")
        nc.sync.dma_start(bpack, bp_d)
        wqq = [wpack[:, ts(c, 128)] for c in range(4)]
        wkv_e = [wpack[:, ds(512 + c * 128, 128)] for c in range(4)]
        wkv_o = [wpack[:, ds(1024 + c * 128, 128)] for c in range(4)]
        bq_sb = bpack[:, 0:1]
        bk_sb = bpack[:, 1:2]
        bv_sb = bpack[:, 2:66]

        ident = singles.tile([128, 128], F32, name="ident", tag="ident")
        make_identity(nc, ident)
        identb = singles.tile([128, 128], BF16, name="identb", tag="identb")
        make_identity(nc, identb)

        # warm the exp table while DMAs stream
        warm = singles.tile([64, 1], F32, name="warm", tag="warm")
        nc.scalar.activation(warm, bpack[0:64, 0:1], EXP, scale=1.0)

        # --- activation SBUF tensors (filled by the DMAs below) ---
        yt = [
            singles.tile([128, NH], BF16, name=f"yt{c}", tag=f"yt{c}")
            for c in range(4)
        ]
        zt = [
            [
                singles.tile([128, 2048], BF16, name=f"zt{c}{h}", tag=f"zt{c}{h}")
                for h in range(2)
            ]
            for c in range(4)
        ]
        qt_blk = [
            singles.tile([128, 512], BF16, name=f"qt{j}", tag=f"qt{j}")
            for j in range(QBLK)
        ]
        kt_blk = [
            singles.tile([128, 512], BF16, name=f"kt{j}", tag=f"kt{j}")
            for j in range(4)
        ]
        v_sb = [
            singles.tile([128, KQ + 1], BF16, name=f"v{t}", tag=f"v{t}")
            for t in range(NT)
        ]

        # --- all input DMAs issued upfront, spread over 3 issuing engines ---
        # z half/piece granularity: [128, 1024] per DMA
        for h in range(2):
            for pc in range(2):
                for c in range(4):
                    eng = nc.sync if c % 2 == 0 else nc.scalar
                    eng.dma_start(
                        zt[c][h][:, ds(pc * 1024, 1024)],
                        zt_d[c, :, ds(h * 2048 + pc * 1024, 1024)],
                    )
            for c in range(4):
                nc.gpsimd.dma_start(
                    yt[c][:, ds(h * 1024, 1024)],
                    yt_d[c, :, ds(h * 1024, 1024)],
                )

        def q_proj(j):
            # [wq|wq] stationary -> Q^T appears on both partition halves
            q_ps = ppool.tile([128, 512], F32, name="proj", tag="proj")
            for c in range(4):
                nc.tensor.matmul(
                    q_ps,
                    lhsT=wqq[c],
                    rhs=yt[c][:, ts(j, 512)],
                    start=(c == 0),
                    stop=(c == 3),
                )
            nc.vector.tensor_scalar_add(qt_blk[j], q_ps, bq_sb)

        def kv_proj(b):
            # one pass over z m-block b (tiles 4b..4b+3) produces
            # K^T -> kt_blk[b//2] (even b: partitions 0:64, odd: 64:128)
            # V^T -> staging, then PE-transposed into v_sb tiles
            h = b // 4
            bb = b % 4
            odd = b % 2
            w = wkv_o if odd else wkv_e
            khalf = slice(64, 128) if odd else slice(0, 64)
            vhalf = slice(0, 64) if odd else slice(64, 128)
            kv_ps = ppool.tile([128, 512], F32, name="proj", tag="proj")
            for c in range(4):
                nc.tensor.matmul(
                    kv_ps,
                    lhsT=w[c],
                    rhs=zt[c][h][:, ts(bb, 512)],
                    start=(c == 0),
                    stop=(c == 3),
                )
            nc.vector.tensor_scalar_add(
                kt_blk[b // 2][khalf, :], kv_ps[khalf, :], bk_sb[khalf, :]
            )
            vt_sb = vtpool.tile([128, 512], BF16, name="vt", tag="vt")
            nc.vector.tensor_copy(vt_sb[vhalf, :], kv_ps[vhalf, :])
            vib = identb[64:128, 64:128] if odd == 0 else identb[0:64, 0:64]
            for i in range(4):
                t = 4 * b + i
                v_ps = ppool.tile([128, 512], BF16, name="projb", tag="proj")
                nc.tensor.matmul(
                    v_ps[:, 0:KQ],
                    lhsT=vt_sb[vhalf, ts(i, 128)],
                    rhs=vib,
                    is_transpose=True,
                    start=True,
                    stop=True,
                    tile_position=(vhalf.start, 0),
                )
                nc.vector.tensor_copy(v_sb[t][:, 0:KQ], v_ps[:, 0:KQ])
                nc.vector.memset(v_sb[t][:, KQ : KQ + 1], 1.0)

        q_proj(0)
        kv_proj(0)
        kv_proj(1)
        q_proj(1)
        kv_proj(2)
        kv_proj(3)
        q_proj(2)
        kv_proj(4)
        kv_proj(5)
        q_proj(3)
        kv_proj(6)
        kv_proj(7)

        # --- attention loop ---
        for nb in range(NBLK):
            o_ps = opool.tile([128, 512], F32, name="o", tag="o")
            qlo = qt_blk[nb][0:64, :]
            qhi = qt_blk[nb][64:128, :]
            for p in range(NPAIR):
                jj, col = divmod(p, 4)
                s_ps = spool.tile([128, 1024], F32, name="s", tag="s")
                nc.tensor.matmul(
                    s_ps[:, 0:512],
                    lhsT=kt_blk[jj][0:64, ts(col, 128)],
                    rhs=qlo,
                    start=True,
                    stop=True,
                    tile_position=(0, 0),
                )
                nc.tensor.matmul(
                    s_ps[:, 512:1024],
                    lhsT=kt_blk[jj][64:128, ts(col, 128)],
                    rhs=qhi,
                    start=True,
                    stop=True,
                    tile_position=(64, 0),
                )
                e_t = epool.tile([128, 1024], BF16, name="e", tag="e")
                nc.scalar.activation(e_t, s_ps, EXP, scale=SCALE)
                nc.tensor.matmul(
                    o_ps[0:65, :],
                    lhsT=v_sb[8 * jj + col],
                    rhs=e_t[:, 0:512],
                    start=(p == 0),
                    stop=False,
                )
                nc.tensor.matmul(
                    o_ps[0:65, :],
                    lhsT=v_sb[8 * jj + 4 + col],
                    rhs=e_t[:, 512:1024],
                    start=False,
                    stop=(p == NPAIR - 1),
                )

            # --- finalize this n-block: transpose, normalize+bias, store ---
            ot_sb = otpool.tile([128, 512], F32, name="ot", tag="ot")
            for s in range(4):
                nc.vector.tensor_copy(
                    ot_sb[0:65, ts(s, 128)], o_ps[0:65, ts(s, 128)]
                )
                ot_ps = ppool.tile([128, 512], F32, name="proj", tag="proj")
                nc.tensor.matmul(
                    ot_ps[:, 0:65],
                    lhsT=ot_sb[0:65, ts(s, 128)],
                    rhs=ident[0:65, 0:65],
                    is_transpose=True,
                    start=True,
                    stop=True,
                )
                rcp = rpool.tile([128, 1], F32, name="rcp", tag="rcp")
                nc.vector.reciprocal(rcp, ot_ps[:, 64:65])
                o_st = ostage.tile([128, KQ], F32, name="ost", tag="ost")
                nc.vector.scalar_tensor_tensor(
                    out=o_st,
                    in0=ot_ps[:, 0:KQ],
                    scalar=rcp,
                    in1=bv_sb,
                    op0=MULT,
                    op1=ADD,
                )
                nc.sync.dma_start(o_d[ds(nb * 512 + s * 128, 128), :], o_st)

    nc.compile()
    return nc


def _get_program():
    if "nc" not in _CACHE:
        _CACHE["nc"] = build_program()
    return _CACHE["nc"]


def make_in_maps(z, y, Wq, bq, Wk, bk, Wv, bv):
    bf16 = ml_dtypes.bfloat16
    zt = np.ascontiguousarray(z.astype(bf16).transpose(0, 2, 1))  # [B, 512, M]
    yt = np.ascontiguousarray(y.astype(bf16).transpose(0, 2, 1))  # [B, 512, N]
    wq = Wq.astype(bf16).reshape(4, 128, KQ)
    wk = Wk.astype(bf16).reshape(4, 128, KQ)
    wv = Wv.astype(bf16).reshape(4, 128, KQ)
    wpack = np.empty((128, 1536), dtype=bf16)
    for c in range(4):
        wpack[:, c * 128 : c * 128 + 64] = wq[c]
        wpack[:, c * 128 + 64 : c * 128 + 128] = wq[c]
        wpack[:, 512 + c * 128 : 512 + c * 128 + 64] = wk[c]
        wpack[:, 512 + c * 128 + 64 : 512 + c * 128 + 128] = wv[c]
        wpack[:, 1024 + c * 128 : 1024 + c * 128 + 64] = wv[c]
        wpack[:, 1024 + c * 128 + 64 : 1024 + c * 128 + 128] = wk[c]
    bpack = np.empty((128, 66), dtype=np.float32)
    bpack[0:64, 0] = bq
    bpack[64:128, 0] = bq
    bpack[0:64, 1] = bk
    bpack[64:128, 1] = bk
    bpack[:, 2:66] = bv[None, :]
    in_maps = []
    for c in range(NCORES):
        b, h = divmod(c, 2)
        in_maps.append(
            {
                "zt": zt[b].reshape(4, 128, M),
                "yt": np.ascontiguousarray(
                    yt[b][:, h * NH : (h + 1) * NH]
                ).reshape(4, 128, NH),
                "wpack": wpack,
                "bpack": bpack,
            }
        )
    return in_maps


def kernel(z, y, Wq, bq, Wk, bk, Wv, bv):
    from concourse import bass_utils

    nc = _get_program()
    in_maps = make_in_maps(z, y, Wq, bq, Wk, bk, Wv, bv)
    res = bass_utils.run_bass_kernel_spmd(nc, in_maps, core_ids=list(range(NCORES)))
    out = np.empty((B, N, KQ), dtype=np.float32)
    for c in range(NCORES):
        b, h = divmod(c, 2)
        out[b, h * NH : (h + 1) * NH, :] = res.results[c]["o"]
    return out
